# revision 1
# baseline (speedup 1.0000x reference)
"""Multi-Head Latent Attention (DeepSeek-style MLA) forward on 8 Trainium2 cores.

Sharding: data-parallel over batch (4) x tensor-parallel over heads (2 groups
of 8). Core c handles batch c//2, head-group c%2. The o_proj row-shard
partial sums are reduced on host (cheap f32 add), along with b_o.

Device layout strategy: everything is computed "feature-major" (transposed)
so the TensorE contraction dim always sits on SBUF partitions:
  xT [DM, S] -> cT = W_d.T @ xT -> q/k feature-major, v seq-major,
  scores computed transposed (sT[t, q]) so softmax normalization arrives
  for free via an appended ones-column on V, and no transposes are needed
  anywhere. Softmax denominators are divided out at PV-eviction time via a
  GpSimd partition_broadcast of the reciprocal row.

All matmuls run in bf16 with fp32 PSUM accumulation; softmax (exp, masks,
reciprocal) in fp32. Validated end-to-end rel err vs the fp32 reference:
~4.7e-3 (Frobenius).
"""

import os
import numpy as np
import ml_dtypes

BF = ml_dtypes.bfloat16

B, S, DM, DE, H, DH, DC, DCq, DR = 4, 2048, 1024, 1024, 16, 64, 128, 128, 32
HL = H // 2            # heads per core
DEL = HL * DH          # 512: per-core up-proj width
DRL = HL * DR          # 256: per-core rope-q width
SCALE = 1.0 / float(np.sqrt(DH + DR))
P = 128
KD = DM // P           # 8 contraction chunks for down projections
NT = S // P            # 16 key chunks
NQ = S // 512          # 4 query chunks of 512
NS = S // 512          # 4 s-splits for projections
TGRP = 3               # scores-psum group size (t-chunks per exp op)

_CACHE: dict = {}


def _build_program():
    import concourse.mybir as mybir
    import concourse.tile as tile
    from concourse import bacc
    from contextlib import ExitStack

    fp32 = mybir.dt.float32
    bf16 = mybir.dt.bfloat16
    MUL = mybir.AluOpType.mult
    ADD = mybir.AluOpType.add
    EXP = mybir.ActivationFunctionType.Exp

    nc = bacc.Bacc("TRN2", target_bir_lowering=False, debug=False)

    d = {}

    def din(name, shape, dt=bf16):
        d[name] = nc.dram_tensor(name, list(shape), dt, kind="ExternalInput").ap()

    din("xT", (DM, S))
    din("W_dq", (DM, DC)); din("W_dkv", (DM, DC)); din("W_kr", (DM, DR))
    din("W_uq", (DCq, DEL)); din("W_uk", (DC, DEL)); din("W_uv", (DC, DEL))
    din("W_qr", (DCq, DRL)); din("W_o", (DEL, DM))
    din("b_dq", (1, DC)); din("b_dkv", (1, DC)); din("b_kr", (1, DR))
    din("b_uq", (1, DEL)); din("b_uk", (1, DEL)); din("b_uv", (1, DEL))
    din("b_qr", (1, DRL))
    din("cosq", (P, S), fp32); din("sinqs", (P, S), fp32)
    din("maskT", (P, 4 * 512))
    out_ap = nc.dram_tensor("out", [S, DM], fp32, kind="ExternalOutput").ap()

    swap32 = [p ^ 1 for p in range(32)]

    with tile.TileContext(nc) as tc:
        with ExitStack() as root:
            const = root.enter_context(tc.tile_pool(name="const", bufs=1))

            # ---- resident constants ----
            w_dq = const.tile([P, KD * DC], bf16, name="w_dq")
            nc.sync.dma_start(w_dq[:].rearrange("p (k c) -> p k c", c=DC),
                              d["W_dq"].rearrange("(k p) c -> p k c", p=P))
            w_dkv = const.tile([P, KD * DC], bf16, name="w_dkv")
            nc.sync.dma_start(w_dkv[:].rearrange("p (k c) -> p k c", c=DC),
                              d["W_dkv"].rearrange("(k p) c -> p k c", p=P))
            w_kr = const.tile([P, KD * DR], bf16, name="w_kr")
            nc.sync.dma_start(w_kr[:].rearrange("p (k c) -> p k c", c=DR),
                              d["W_kr"].rearrange("(k p) c -> p k c", p=P))
            w_uq = const.tile([P, DEL], bf16, name="w_uq")
            nc.sync.dma_start(w_uq[:], d["W_uq"])
            w_uk = const.tile([P, DEL], bf16, name="w_uk")
            nc.sync.dma_start(w_uk[:], d["W_uk"])
            w_uv = const.tile([P, DEL], bf16, name="w_uv")
            nc.sync.dma_start(w_uv[:], d["W_uv"])
            w_qr = const.tile([P, DRL], bf16, name="w_qr")
            nc.sync.dma_start(w_qr[:], d["W_qr"])
            w_o = const.tile([P, 4 * DM], bf16, name="w_o")
            nc.sync.dma_start(w_o[:].rearrange("p (e n) -> p e n", n=DM),
                              d["W_o"].rearrange("(e p) n -> p e n", p=P))
            cosq = const.tile([P, S], fp32, name="cosq")
            nc.sync.dma_start(cosq[:], d["cosq"])
            sinqs = const.tile([P, S], fp32, name="sinqs")
            nc.sync.dma_start(sinqs[:], d["sinqs"])
            maskt = const.tile([P, 4 * 512], bf16, name="maskt")
            nc.sync.dma_start(maskt[:], d["maskT"])

            btiles = {}
            for bn, bw in [("b_dq", DC), ("b_dkv", DC), ("b_kr", DR),
                           ("b_uq", DEL), ("b_uk", DEL), ("b_uv", DEL),
                           ("b_qr", DRL)]:
                bt = const.tile([1, bw], bf16, name=f"t{bn}")
                nc.sync.dma_start(bt[:], d[bn])
                btiles[bn] = bt
            ones_row = const.tile([1, 512], bf16, name="ones_row")
            nc.vector.memset(ones_row[:], 1.0)
            ones_col = const.tile([1, P], bf16, name="ones_col")
            nc.vector.memset(ones_col[:], 1.0)

            # ---- persistent activations ----
            acts = root.enter_context(tc.tile_pool(name="acts", bufs=1))
            c_q = acts.tile([P, S], bf16, name="c_q")
            c_kv = acts.tile([P, S], bf16, name="c_kv")
            q_t = [acts.tile([P, S], bf16, name=f"q_t{h}") for h in range(HL)]
            k_t = [acts.tile([P, S], bf16, name=f"k_t{h}") for h in range(HL)]
            vt = [acts.tile([P, HL * (DH + 1)], bf16, name=f"v{i}") for i in range(NT)]
            attn = [acts.tile([P, S], bf16, name=f"attn{e}") for e in range(4)]

            # ================= Phase A: down projections =================
            with ExitStack() as phAB:
                xp = phAB.enter_context(tc.tile_pool(name="xTp", bufs=1))
                xt = xp.tile([P, KD * S], bf16, name="xt")
                nc.sync.dma_start(xt[:].rearrange("p (k s) -> p k s", s=S),
                                  d["xT"].rearrange("(k p) s -> p k s", p=P))

                rope_src = phAB.enter_context(tc.tile_pool(name="ropesrc", bufs=1))
                q_rr = [rope_src.tile([P, S], bf16, name=f"q_rr{r}") for r in range(2)]
                k_rr = rope_src.tile([32, S], bf16, name="k_rr")

                pps = phAB.enter_context(
                    tc.tile_pool(name="proj_psum", bufs=4, space="PSUM"))
                tmp = phAB.enter_context(tc.tile_pool(name="rope_tmp", bufs=4))

                def rope(psrc, rows, ns, dest):
                    """dest[:rows, ns*512:+512] = rope(psrc[:rows]) (psum f32 -> bf16)."""
                    sl = slice(ns * 512, (ns + 1) * 512)
                    t1 = tmp.tile([P, 512], fp32, name="rt1", tag="rt1")
                    nc.vector.tensor_tensor(t1[:rows, :], psrc[:rows, :],
                                            cosq[0:rows, sl], MUL)
                    t2 = tmp.tile([P, 512], fp32, name="rt2", tag="rt2")
                    nc.vector.stream_shuffle(t2[:rows, :], psrc[:rows, :], swap32)
                    nc.vector.tensor_tensor(t2[:rows, :], t2[:rows, :],
                                            sinqs[0:rows, sl], MUL)
                    nc.vector.tensor_tensor(dest[0:rows, sl], t1[:rows, :],
                                            t2[:rows, :], ADD)

                for ns in range(NS):
                    ssl = slice(ns * 512, (ns + 1) * 512)
                    for pname, wtile, bname, mrows in [
                            ("dq", w_dq, "b_dq", P),
                            ("dkv", w_dkv, "b_dkv", P),
                            ("kr", w_kr, "b_kr", DR)]:
                        pt = pps.tile([P, 512], fp32, name=f"ps_{pname}{ns}",
                                      tag="projps")
                        nc.tensor.matmul(pt[0:mrows, :], btiles[f"b_{pname}"][:],
                                         ones_row[:], start=True, stop=False)
                        for k in range(KD):
                            nc.tensor.matmul(
                                pt[0:mrows, :],
                                wtile[:, k * mrows:(k + 1) * mrows],
                                xt[:, k * S + ns * 512: k * S + (ns + 1) * 512],
                                start=False, stop=(k == KD - 1))
                        if pname == "dq":
                            nc.vector.tensor_copy(c_q[:, ssl], pt[:])
                        elif pname == "dkv":
                            nc.vector.tensor_copy(c_kv[:, ssl], pt[:])
                        else:
                            rope(pt, DR, ns, k_rr)

                # ================= Phase B: up projections =================
                for e in range(4):
                    esl = slice(e * P, (e + 1) * P)
                    for ns in range(NS):
                        ssl = slice(ns * 512, (ns + 1) * 512)
                        pq = pps.tile([P, 512], fp32, name=f"ps_uq{e}{ns}",
                                      tag="projps")
                        nc.tensor.matmul(pq[:], btiles["b_uq"][0:1, esl],
                                         ones_row[:], start=True, stop=False)
                        nc.tensor.matmul(pq[:], w_uq[:, esl], c_q[:, ssl],
                                         start=False, stop=True)
                        nc.vector.tensor_copy(q_t[2 * e][0:64, ssl], pq[0:64, :])
                        nc.vector.tensor_copy(q_t[2 * e + 1][0:64, ssl], pq[64:P, :])

                        pk = pps.tile([P, 512], fp32, name=f"ps_uk{e}{ns}",
                                      tag="projps")
                        nc.tensor.matmul(pk[:], btiles["b_uk"][0:1, esl],
                                         ones_row[:], start=True, stop=False)
                        nc.tensor.matmul(pk[:], w_uk[:, esl], c_kv[:, ssl],
                                         start=False, stop=True)
                        nc.vector.tensor_copy(k_t[2 * e][0:64, ssl], pk[0:64, :])
                        nc.vector.tensor_copy(k_t[2 * e + 1][0:64, ssl], pk[64:P, :])

                for r in range(2):
                    rsl = slice(r * P, (r + 1) * P)
                    for ns in range(NS):
                        pr = pps.tile([P, 512], fp32, name=f"ps_qr{r}{ns}",
                                      tag="projps")
                        nc.tensor.matmul(pr[:], btiles["b_qr"][0:1, rsl],
                                         ones_row[:], start=True, stop=False)
                        nc.tensor.matmul(pr[:], w_qr[:, rsl],
                                         c_q[:, ns * 512:(ns + 1) * 512],
                                         start=False, stop=True)
                        rope(pr, P, ns, q_rr[r])

                for it in range(NT):
                    pv = pps.tile([P, 512], fp32, name=f"ps_v{it}", tag="projps")
                    nc.tensor.matmul(pv[:], ones_col[:], btiles["b_uv"][:],
                                     start=True, stop=False)
                    nc.tensor.matmul(pv[:], c_kv[:, it * P:(it + 1) * P],
                                     w_uv[:], start=False, stop=True)
                    g = vt[it][:].rearrange("p (h c) -> p h c", c=DH + 1)
                    nc.vector.tensor_copy(
                        g[:, :, 0:DH],
                        pv[:].rearrange("p (h c) -> p h c", c=DH))
                    nc.vector.memset(g[:, :, DH:DH + 1], 1.0)

                # assemble rope rows into q_t / k_t via SB->SB DMA
                for h in range(HL):
                    nc.sync.dma_start(
                        q_t[h][64:96, :],
                        q_rr[h // 4][(h % 4) * 32:(h % 4) * 32 + 32, :])
                    nc.sync.dma_start(k_t[h][64:96, :], k_rr[0:32, :])

            # ================= Phase C: attention =================
            with ExitStack() as phC:
                scp = phC.enter_context(
                    tc.tile_pool(name="sc_psum", bufs=2, space="PSUM"))
                pvp = phC.enter_context(
                    tc.tile_pool(name="pv_psum", bufs=2, space="PSUM"))
                ppool = phC.enter_context(tc.tile_pool(name="ptiles", bufs=3))
                rpool = phC.enter_context(tc.tile_pool(name="recips", bufs=3))

                for h in range(HL):
                    e, half = h // 2, h % 2
                    for jq in range(NQ):
                        qsl = slice(jq * 512, (jq + 1) * 512)
                        n_t = 4 * jq + 4
                        pvacc = pvp.tile([65, 512], fp32, name=f"pva{h}{jq}",
                                         tag="pva")
                        mm = 0
                        for g0 in range(0, n_t, TGRP):
                            cnt = min(TGRP, n_t - g0)
                            w = cnt * 512
                            sc = scp.tile([P, TGRP * 512], fp32, name=f"sc{h}{jq}{g0}",
                                          tag="sc")
                            for ci in range(cnt):
                                it = g0 + ci
                                nc.tensor.matmul(
                                    sc[:, ci * 512:(ci + 1) * 512],
                                    k_t[h][0:96, it * P:(it + 1) * P],
                                    q_t[h][0:96, qsl], start=True, stop=True)
                            pt = ppool.tile([P, TGRP * 512], bf16, name=f"p{h}{jq}{g0}",
                                            tag="pt")
                            nc.scalar.activation(pt[:, :w], sc[:, :w], EXP,
                                                 scale=SCALE)
                            for ci in range(cnt):
                                it = g0 + ci
                                dlt = it - 4 * jq
                                psl = slice(ci * 512, (ci + 1) * 512)
                                if dlt >= 0:
                                    nc.vector.tensor_tensor(
                                        pt[:, psl], pt[:, psl],
                                        maskt[:, dlt * 512:(dlt + 1) * 512], MUL)
                                nc.tensor.matmul(
                                    pvacc[:], vt[it][:, h * (DH + 1):(h + 1) * (DH + 1)],
                                    pt[:, psl], start=(mm == 0), stop=(mm == n_t - 1))
                                mm += 1
                        rc = rpool.tile([1, 512], fp32, name=f"rc{h}{jq}", tag="rc")
                        nc.vector.reciprocal(rc[:], pvacc[64:65, :])
                        rbc = rpool.tile([64, 512], fp32, name=f"rbc{h}{jq}",
                                         tag="rbc")
                        nc.gpsimd.partition_broadcast(rbc[:], rc[:])
                        nc.vector.tensor_tensor(
                            attn[e][half * 64:half * 64 + 64, qsl],
                            pvacc[0:64, :], rbc[:], MUL)

            # ================= Phase D: output projection =================
            with ExitStack() as phD:
                ops = phD.enter_context(
                    tc.tile_pool(name="o_psum", bufs=2, space="PSUM"))
                osb = phD.enter_context(tc.tile_pool(name="o_sb", bufs=2))
                for m in range(NT):
                    ot = osb.tile([P, DM], fp32, name=f"o{m}", tag="osb")
                    for half in range(2):
                        po = ops.tile([P, 512], fp32, name=f"po{m}{half}", tag="po")
                        for e in range(4):
                            nc.tensor.matmul(
                                po[:], attn[e][:, m * P:(m + 1) * P],
                                w_o[:, e * DM + half * 512: e * DM + half * 512 + 512],
                                start=(e == 0), stop=(e == 3))
                        nc.vector.tensor_copy(ot[:, half * 512:(half + 1) * 512],
                                              po[:])
                    nc.sync.dma_start(out_ap[m * P:(m + 1) * P, :], ot[:])

    nc.compile()
    return nc


def _host_tables():
    inv = 1.0 / (10000.0 ** (np.arange(0, DR, 2, dtype=np.float32) / DR))
    t = np.arange(S, dtype=np.float32)
    ang = t[:, None] * inv[None, :].astype(np.float32)
    cos = np.cos(ang).astype(np.float32).T    # [16, S]
    sin = np.sin(ang).astype(np.float32).T
    pair = (np.arange(P) % DR) >> 1
    cosq = np.ascontiguousarray(cos[pair, :])               # [128, S]
    sinq = sin[pair, :]
    sign = np.where(np.arange(P) % 2 == 0, -1.0, 1.0).astype(np.float32)
    sinqs = np.ascontiguousarray(sinq * sign[:, None])
    tloc = np.arange(P)[:, None]
    qloc = np.arange(512)[None, :]
    mask = np.concatenate(
        [(tloc + P * dd <= qloc) for dd in range(4)], axis=1).astype(BF)
    return cosq, sinqs, np.ascontiguousarray(mask)


def _in_maps(inputs):
    x = np.asarray(inputs["x"], np.float32)
    cosq, sinqs, mask = _host_tables()
    shared = {
        "W_dq": np.asarray(inputs["W_dq"], np.float32).astype(BF),
        "W_dkv": np.asarray(inputs["W_dkv"], np.float32).astype(BF),
        "W_kr": np.asarray(inputs["W_kr"], np.float32).astype(BF),
        "b_dq": np.asarray(inputs["b_dq"], np.float32).astype(BF)[None, :],
        "b_dkv": np.asarray(inputs["b_dkv"], np.float32).astype(BF)[None, :],
        "b_kr": np.asarray(inputs["b_kr"], np.float32).astype(BF)[None, :],
        "cosq": cosq, "sinqs": sinqs, "maskT": mask,
    }
    grp = []
    for g in range(2):
        ge = slice(g * DEL, (g + 1) * DEL)
        gr = slice(g * DRL, (g + 1) * DRL)
        grp.append({
            "W_uq": np.ascontiguousarray(np.asarray(inputs["W_uq"], np.float32)[:, ge]).astype(BF),
            "W_uk": np.ascontiguousarray(np.asarray(inputs["W_uk"], np.float32)[:, ge]).astype(BF),
            "W_uv": np.ascontiguousarray(np.asarray(inputs["W_uv"], np.float32)[:, ge]).astype(BF),
            "W_qr": np.ascontiguousarray(np.asarray(inputs["W_qr"], np.float32)[:, gr]).astype(BF),
            "W_o": np.ascontiguousarray(np.asarray(inputs["W_o"], np.float32)[ge, :]).astype(BF),
            "b_uq": np.asarray(inputs["b_uq"], np.float32)[None, ge].astype(BF),
            "b_uk": np.asarray(inputs["b_uk"], np.float32)[None, ge].astype(BF),
            "b_uv": np.asarray(inputs["b_uv"], np.float32)[None, ge].astype(BF),
            "b_qr": np.asarray(inputs["b_qr"], np.float32)[None, gr].astype(BF),
        })
    maps = []
    for c in range(8):
        b, g = divmod(c, 2)
        m = dict(shared)
        m.update(grp[g])
        m["xT"] = np.ascontiguousarray(x[b].T).astype(BF)
        maps.append(m)
    return maps


def _combine(results, inputs):
    b_o = np.asarray(inputs["b_o"], np.float32)
    outs = []
    for b in range(B):
        o = (results[2 * b]["out"].astype(np.float32)
             + results[2 * b + 1]["out"].astype(np.float32) + b_o)
        outs.append(o)
    return np.stack(outs).astype(np.float32)


def kernel(**inputs):
    from concourse.bass_utils import run_bass_kernel_spmd
    if "nc" not in _CACHE:
        _CACHE["nc"] = _build_program()
    nc = _CACHE["nc"]
    maps = _in_maps(inputs)
    trace = bool(int(os.environ.get("KERNEL_TRACE", "0")))
    res = run_bass_kernel_spmd(nc, maps, list(range(8)), trace=trace)
    _CACHE["last_result"] = res
    return _combine(res.results, inputs)
